# revision 6
# baseline (speedup 1.0000x reference)
"""GraphConv (DeepChem) Bass kernel for 8 Trainium2 NeuronCores.

Sharding: data-parallel over rows within each degree bucket. Each core owns
1/8 of every bucket (deg0: 1500 rows, deg1-10: 3750 rows each). W/b
replicated.

Host-side prep (pure layout, no arithmetic): for each core the replicated
node_features table is re-laid-out into the transposed per-(tile,
neighbor-slot) feature stream the device consumes — bf16 blocks
[din, row], j-major within batches of 4 tiles so each neighbor slot is one
N=512 matmul. (Device-side per-row gathers wall on the Pool engine's SWDGE
descriptor generation: ~1us/call for INDIRECT1D x 1650 calls, or ~7.4ns/idx
Q7 time for batched dma_gather — both >= 1.6ms/core. Streaming the
host-materialized layout keeps all 16 SDMA engines at line rate with large
descriptors and leaves Pool idle.)

Device algorithm per batch of 4 tiles of degree d:
  - ONE contiguous HWDGE load (stream slice [128, 4*(d+1)*128], j-major)
  - PE: psum[128, 512] accumulates sum_j W[2d-1]^T @ Gj^T (one N=512
    matmul per slot j) + W[2d]^T @ selfT, fp32
  - DVE eviction adds bias (per-partition scalar) -> bf16 store [128, 512]
Host un-transposes, upcasts, and re-concatenates bucket shards.
"""
import os
import sys
import types
import numpy as np
import ml_dtypes

import concourse.bacc as bacc
import concourse.mybir as mybir
import concourse.tile as tile
from concourse.bass_utils import run_bass_kernel_spmd

N_DEG0 = 12000
N_PER_DEG = 30000
MAX_DEG = 10
D = 128
N_NODES = N_DEG0 + MAX_DEG * N_PER_DEG  # 312000
N_PARAMS = 2 * MAX_DEG + 1  # 21
N_CORES = 8

C_DEG0 = N_DEG0 // N_CORES          # 1500
C_DEG = N_PER_DEG // N_CORES        # 3750
P_DEG0 = 1536                       # padded to 12 tiles of 128
P_DEG = 3840                        # padded to 30 tiles of 128
T_DEG0 = P_DEG0 // 128              # 12
T_DEG = P_DEG // 128                # 30
LOCAL_COLS = P_DEG0 + MAX_DEG * P_DEG  # 39936 local rows per core

GB = 4                              # tiles per batch (PSUM bank = 512 fp32)

BF16 = mybir.dt.bfloat16
NP_BF16 = ml_dtypes.bfloat16


def _plan(ntiles):
    plan = []
    left = ntiles
    while left > 0:
        b = min(GB, left)
        plan.append(b)
        left -= b
    return plan


# stream column base per degree (deg d tile = d+1 blocks of 128 cols)
STRM_BASE = {0: 0}
_off = P_DEG0
for _d in range(1, MAX_DEG + 1):
    STRM_BASE[_d] = _off
    _off += T_DEG * (_d + 1) * 128
STRM_COLS = _off                    # 251136

_COMPILED = None
LAST_RESULT = None


def _maybe_install_trace_hook():
    """Inject antenv.axon_hooks so trace=True can NTFF-profile under axon."""
    try:
        import antenv.axon_hooks  # noqa: F401
        return True
    except ImportError:
        pass
    try:
        hooks = types.ModuleType("antenv.axon_hooks")
        hooks._hook = None

        def _set(h):
            hooks._hook = h

        def _get():
            return hooks._hook

        hooks.set_axon_ntff_profile_hook = _set
        hooks.get_axon_ntff_profile_hook = _get
        sys.modules["antenv.axon_hooks"] = hooks
        import antenv

        antenv.axon_hooks = hooks
        from trn_agent_boot.trn_boot import _ntff_profile_via_ctypes

        _set(_ntff_profile_via_ctypes("/opt/axon/libaxon_pjrt.so"))
        return True
    except Exception:
        return False


def _build():
    nc = bacc.Bacc()
    strm = nc.declare_dram_parameter(
        "strm", [D, STRM_COLS], BF16, isOutput=False
    )
    w_in = nc.declare_dram_parameter(
        "w", [128, N_PARAMS * 128], BF16, isOutput=False
    )
    bsumT = nc.declare_dram_parameter(
        "bsumT", [D, MAX_DEG + 1], mybir.dt.float32, isOutput=False
    )
    outT = nc.declare_dram_parameter(
        "outT", [D, LOCAL_COLS], BF16, isOutput=True
    )

    with tile.TileContext(nc) as tc:
        with (
            tc.tile_pool(name="const", bufs=1) as constp,
            tc.tile_pool(name="gp", bufs=4) as gp,
            tc.tile_pool(name="obp", bufs=4) as obp,
            tc.tile_pool(name="psout", bufs=4, space="PSUM") as psout,
        ):
            w_sb = constp.tile([128, N_PARAMS * 128], BF16)
            nc.sync.dma_start(out=w_sb[:], in_=w_in[:, :])
            bs_sb = constp.tile([128, MAX_DEG + 1], mybir.dt.float32)
            nc.sync.dma_start(out=bs_sb[:], in_=bsumT[:, :])

            def do_batch(d, t0, B):
                nblk = d + 1 if d > 0 else 1
                cols0 = STRM_BASE[d] + t0 * nblk * 128
                ncols = B * nblk * 128
                w = B * 128
                g = gp.tile([128, ncols], BF16, tag="g")
                nc.sync.dma_start(out=g[:], in_=strm[:, cols0:cols0 + ncols])
                ob = obp.tile([128, w], BF16, tag="ob")
                ps = psout.tile([128, w], mybir.dt.float32, tag="ps")
                if d > 0:
                    for j in range(d):
                        nc.tensor.matmul(
                            out=ps[:],
                            lhsT=w_sb[:, (2 * d - 1) * 128:(2 * d) * 128],
                            rhs=g[:, j * w:(j + 1) * w],
                            start=(j == 0),
                            stop=False,
                        )
                    nc.tensor.matmul(
                        out=ps[:],
                        lhsT=w_sb[:, (2 * d) * 128:(2 * d + 1) * 128],
                        rhs=g[:, d * w:(d + 1) * w],
                        start=False,
                        stop=True,
                    )
                else:
                    nc.tensor.matmul(
                        out=ps[:],
                        lhsT=w_sb[:, 0:128],
                        rhs=g[:, 0:w],
                        start=True,
                        stop=True,
                    )
                nc.vector.tensor_scalar_add(
                    out=ob[:], in0=ps[:], scalar1=bs_sb[:, d:d + 1]
                )
                base = 0 if d == 0 else P_DEG0 + (d - 1) * P_DEG
                oc0 = base + t0 * 128
                nc.sync.dma_start(out=outT[:, oc0:oc0 + w], in_=ob[:])

            t0 = 0
            for B in _plan(T_DEG0):
                do_batch(0, t0, B)
                t0 += B
            for d in range(1, MAX_DEG + 1):
                t0 = 0
                for B in _plan(T_DEG):
                    do_batch(d, t0, B)
                    t0 += B

    nc.compile()
    return nc


def kernel(node_features, deg_slice, adj1, adj2, adj3, adj4, adj5, adj6,
           adj7, adj8, adj9, adj10, W, b):
    global _COMPILED, LAST_RESULT
    nf32 = np.ascontiguousarray(np.asarray(node_features, dtype=np.float32))
    nf = nf32.astype(NP_BF16)
    adjs = [np.asarray(a, dtype=np.int32)
            for a in (adj1, adj2, adj3, adj4, adj5, adj6, adj7, adj8, adj9, adj10)]
    Wf = np.asarray(W, dtype=np.float32)
    bf = np.asarray(b, dtype=np.float32)

    # weights packed [din, k*128+dout] bf16
    wpack = np.ascontiguousarray(
        Wf.transpose(1, 0, 2).reshape(D, N_PARAMS * D)
    ).astype(NP_BF16)

    # bias pre-sum (affine marshalling): bsum[0]=b[0]; bsum[d]=b[2d-1]+b[2d]
    bsum = np.empty((MAX_DEG + 1, D), np.float32)
    bsum[0] = bf[0]
    for d in range(1, MAX_DEG + 1):
        bsum[d] = bf[2 * d - 1] + bf[2 * d]
    bsumT = np.ascontiguousarray(bsum.T)

    in_maps = []
    for c in range(N_CORES):
        # block row-id list in stream order: [nblk_total, 128] int32.
        # Within each batch of B tiles: j-major — for j in 0..d: B blocks.
        blocks = []
        r0 = np.arange(P_DEG0, dtype=np.int32)
        d0ids = np.where(r0 < C_DEG0, c * C_DEG0 + r0, 0).reshape(T_DEG0, 128)
        t0 = 0
        for B in _plan(T_DEG0):
            blocks.append(d0ids[t0:t0 + B])
            t0 += B
        rd = np.arange(P_DEG, dtype=np.int32)
        for d in range(1, MAX_DEG + 1):
            gs = N_DEG0 + (d - 1) * N_PER_DEG + c * C_DEG
            a = np.zeros((P_DEG, d), np.int32)
            a[:C_DEG] = adjs[d - 1][c * C_DEG:(c + 1) * C_DEG]
            selfid = np.where(rd < C_DEG, gs + rd, 0)
            # per tile: d neighbor blocks + self block -> [T_DEG, d+1, 128]
            tb = np.concatenate(
                [a.reshape(T_DEG, 128, d).transpose(0, 2, 1),
                 selfid.reshape(T_DEG, 1, 128)], axis=1
            )
            t0 = 0
            for B in _plan(T_DEG):
                # j-major within the batch: [d+1, B, 128]
                blocks.append(
                    tb[t0:t0 + B].transpose(1, 0, 2).reshape(-1, 128)
                )
                t0 += B
        R = np.concatenate(blocks, axis=0)  # [1962, 128]
        G = nf[R]                           # [nblk, 128p, 128din]
        strm = np.ascontiguousarray(G.transpose(2, 0, 1)).reshape(D, -1)
        in_maps.append({
            "strm": strm,
            "w": wpack,
            "bsumT": bsumT,
        })

    if _COMPILED is None:
        _COMPILED = _build()

    trace = bool(int(os.environ.get("KERNEL_TRACE", "0")))
    if trace:
        trace = _maybe_install_trace_hook()
    res = run_bass_kernel_spmd(
        _COMPILED, in_maps, core_ids=list(range(N_CORES)), trace=trace
    )
    LAST_RESULT = res

    out = np.empty((N_NODES, D), np.float32)
    for c in range(N_CORES):
        oT = res.results[c]["outT"].astype(np.float32)
        out[c * C_DEG0:(c + 1) * C_DEG0] = oT[:, :C_DEG0].T
        for d in range(1, MAX_DEG + 1):
            base = P_DEG0 + (d - 1) * P_DEG
            gs = N_DEG0 + (d - 1) * N_PER_DEG + c * C_DEG
            out[gs:gs + C_DEG] = oT[:, base:base + C_DEG].T
    return out


# revision 7
# speedup vs baseline: 1.2901x; 1.2901x over previous
"""GraphConv (DeepChem) Bass kernel for 8 Trainium2 NeuronCores.

Sharding: data-parallel over rows within each degree bucket. Each core owns
1/8 of every bucket (deg0: 1500 rows, deg1-10: 3750 rows each). W/b
replicated.

Host-side prep (pure layout, no arithmetic): for each core the replicated
node_features table is re-laid-out into the transposed per-(tile,
neighbor-slot) feature stream the device consumes — bf16 blocks
[din, row], j-major within batches of 4 tiles so each neighbor slot is one
N=512 matmul. (Device-side per-row gathers wall on the Pool engine's SWDGE
descriptor generation: ~1us/call for INDIRECT1D x 1650 calls, or ~7.4ns/idx
Q7 time for batched dma_gather — both >= 1.6ms/core. Streaming the
host-materialized layout keeps all 16 SDMA engines at line rate with large
descriptors and leaves Pool idle.)

Device algorithm per batch of 4 tiles of degree d:
  - ONE contiguous HWDGE load (stream slice [128, 4*(d+1)*128], j-major)
  - PE: psum[128, 512] accumulates sum_j W[2d-1]^T @ Gj^T (one N=512
    matmul per slot j) + W[2d]^T @ selfT, fp32
  - DVE eviction adds bias (per-partition scalar) -> bf16 store [128, 512]
Host un-transposes, upcasts, and re-concatenates bucket shards.
"""
import os
import sys
import types
import numpy as np
import ml_dtypes

import concourse.bacc as bacc
import concourse.mybir as mybir
import concourse.tile as tile
from concourse.bass_utils import run_bass_kernel_spmd

N_DEG0 = 12000
N_PER_DEG = 30000
MAX_DEG = 10
D = 128
N_NODES = N_DEG0 + MAX_DEG * N_PER_DEG  # 312000
N_PARAMS = 2 * MAX_DEG + 1  # 21
N_CORES = 8

C_DEG0 = N_DEG0 // N_CORES          # 1500
C_DEG = N_PER_DEG // N_CORES        # 3750
P_DEG0 = 1536                       # padded to 12 tiles of 128
P_DEG = 3840                        # padded to 30 tiles of 128
T_DEG0 = P_DEG0 // 128              # 12
T_DEG = P_DEG // 128                # 30
LOCAL_COLS = P_DEG0 + MAX_DEG * P_DEG  # 39936 local rows per core

GB = 6                              # tiles per load batch

BF16 = mybir.dt.bfloat16
NP_BF16 = ml_dtypes.bfloat16


def _plan(ntiles):
    plan = []
    left = ntiles
    while left > 0:
        b = min(GB, left)
        plan.append(b)
        left -= b
    return plan


# stream column base per degree (deg d tile = d+1 blocks of 128 cols)
STRM_BASE = {0: 0}
_off = P_DEG0
for _d in range(1, MAX_DEG + 1):
    STRM_BASE[_d] = _off
    _off += T_DEG * (_d + 1) * 128
STRM_COLS = _off                    # 251136

_COMPILED = None
LAST_RESULT = None


def _maybe_install_trace_hook():
    """Inject antenv.axon_hooks so trace=True can NTFF-profile under axon."""
    try:
        import antenv.axon_hooks  # noqa: F401
        return True
    except ImportError:
        pass
    try:
        hooks = types.ModuleType("antenv.axon_hooks")
        hooks._hook = None

        def _set(h):
            hooks._hook = h

        def _get():
            return hooks._hook

        hooks.set_axon_ntff_profile_hook = _set
        hooks.get_axon_ntff_profile_hook = _get
        sys.modules["antenv.axon_hooks"] = hooks
        import antenv

        antenv.axon_hooks = hooks
        from trn_agent_boot.trn_boot import _ntff_profile_via_ctypes

        _set(_ntff_profile_via_ctypes("/opt/axon/libaxon_pjrt.so"))
        return True
    except Exception:
        return False


def _build():
    nc = bacc.Bacc()
    strm = nc.declare_dram_parameter(
        "strm", [D, STRM_COLS], BF16, isOutput=False
    )
    w_in = nc.declare_dram_parameter(
        "w", [128, N_PARAMS * 128], BF16, isOutput=False
    )
    bsumT = nc.declare_dram_parameter(
        "bsumT", [D, MAX_DEG + 1], mybir.dt.float32, isOutput=False
    )
    outT = nc.declare_dram_parameter(
        "outT", [D, LOCAL_COLS], BF16, isOutput=True
    )

    with tile.TileContext(nc) as tc:
        with (
            tc.tile_pool(name="const", bufs=1) as constp,
            tc.tile_pool(name="gp", bufs=6) as gp,
            tc.tile_pool(name="obp", bufs=6) as obp,
            tc.tile_pool(name="psout", bufs=6, space="PSUM") as psout,
        ):
            w_sb = constp.tile([128, N_PARAMS * 128], BF16)
            nc.sync.dma_start(out=w_sb[:], in_=w_in[:, :])
            bs_sb = constp.tile([128, MAX_DEG + 1], mybir.dt.float32)
            nc.sync.dma_start(out=bs_sb[:], in_=bsumT[:, :])

            def do_batch(d, t0, B):
                nblk = d + 1 if d > 0 else 1
                cols0 = STRM_BASE[d] + t0 * nblk * 128
                ncols = B * nblk * 128
                g = gp.tile([128, ncols], BF16, tag="g")
                nc.sync.dma_start(out=g[:], in_=strm[:, cols0:cols0 + ncols])
                ob = obp.tile([128, B * 128], BF16, tag="ob")
                for b in range(B):
                    ps = psout.tile([128, 128], mybir.dt.float32, tag="ps")
                    if d > 0:
                        for j in range(d):
                            c = (b * nblk + j) * 128
                            nc.tensor.matmul(
                                out=ps[:],
                                lhsT=w_sb[:, (2 * d - 1) * 128:(2 * d) * 128],
                                rhs=g[:, c:c + 128],
                                start=(j == 0),
                                stop=False,
                            )
                        c = (b * nblk + d) * 128
                        nc.tensor.matmul(
                            out=ps[:],
                            lhsT=w_sb[:, (2 * d) * 128:(2 * d + 1) * 128],
                            rhs=g[:, c:c + 128],
                            start=False,
                            stop=True,
                        )
                    else:
                        nc.tensor.matmul(
                            out=ps[:],
                            lhsT=w_sb[:, 0:128],
                            rhs=g[:, b * 128:(b + 1) * 128],
                            start=True,
                            stop=True,
                        )
                    nc.vector.tensor_scalar_add(
                        out=ob[:, b * 128:(b + 1) * 128],
                        in0=ps[:],
                        scalar1=bs_sb[:, d:d + 1],
                    )
                base = 0 if d == 0 else P_DEG0 + (d - 1) * P_DEG
                oc0 = base + t0 * 128
                nc.scalar.dma_start(out=outT[:, oc0:oc0 + B * 128], in_=ob[:])

            t0 = 0
            for B in _plan(T_DEG0):
                do_batch(0, t0, B)
                t0 += B
            for d in range(1, MAX_DEG + 1):
                t0 = 0
                for B in _plan(T_DEG):
                    do_batch(d, t0, B)
                    t0 += B

    nc.compile()
    return nc


def kernel(node_features, deg_slice, adj1, adj2, adj3, adj4, adj5, adj6,
           adj7, adj8, adj9, adj10, W, b):
    global _COMPILED, LAST_RESULT
    nf32 = np.ascontiguousarray(np.asarray(node_features, dtype=np.float32))
    nf = nf32.astype(NP_BF16)
    adjs = [np.asarray(a, dtype=np.int32)
            for a in (adj1, adj2, adj3, adj4, adj5, adj6, adj7, adj8, adj9, adj10)]
    Wf = np.asarray(W, dtype=np.float32)
    bf = np.asarray(b, dtype=np.float32)

    # weights packed [din, k*128+dout] bf16
    wpack = np.ascontiguousarray(
        Wf.transpose(1, 0, 2).reshape(D, N_PARAMS * D)
    ).astype(NP_BF16)

    # bias pre-sum (affine marshalling): bsum[0]=b[0]; bsum[d]=b[2d-1]+b[2d]
    bsum = np.empty((MAX_DEG + 1, D), np.float32)
    bsum[0] = bf[0]
    for d in range(1, MAX_DEG + 1):
        bsum[d] = bf[2 * d - 1] + bf[2 * d]
    bsumT = np.ascontiguousarray(bsum.T)

    in_maps = []
    for c in range(N_CORES):
        # block row-id list in stream order: [nblk_total, 128] int32.
        # Within each batch of B tiles: j-major — for j in 0..d: B blocks.
        blocks = []
        r0 = np.arange(P_DEG0, dtype=np.int32)
        d0ids = np.where(r0 < C_DEG0, c * C_DEG0 + r0, 0).reshape(T_DEG0, 128)
        t0 = 0
        for B in _plan(T_DEG0):
            blocks.append(d0ids[t0:t0 + B])
            t0 += B
        rd = np.arange(P_DEG, dtype=np.int32)
        for d in range(1, MAX_DEG + 1):
            gs = N_DEG0 + (d - 1) * N_PER_DEG + c * C_DEG
            a = np.zeros((P_DEG, d), np.int32)
            a[:C_DEG] = adjs[d - 1][c * C_DEG:(c + 1) * C_DEG]
            selfid = np.where(rd < C_DEG, gs + rd, 0)
            # per tile: d neighbor blocks + self block -> [T_DEG, d+1, 128]
            tb = np.concatenate(
                [a.reshape(T_DEG, 128, d).transpose(0, 2, 1),
                 selfid.reshape(T_DEG, 1, 128)], axis=1
            )
            blocks.append(tb.reshape(-1, 128))
        R = np.concatenate(blocks, axis=0)  # [1962, 128]
        G = nf[R]                           # [nblk, 128p, 128din]
        strm = np.ascontiguousarray(G.transpose(2, 0, 1)).reshape(D, -1)
        in_maps.append({
            "strm": strm,
            "w": wpack,
            "bsumT": bsumT,
        })

    if _COMPILED is None:
        _COMPILED = _build()

    trace = bool(int(os.environ.get("KERNEL_TRACE", "0")))
    if trace:
        trace = _maybe_install_trace_hook()
    res = run_bass_kernel_spmd(
        _COMPILED, in_maps, core_ids=list(range(N_CORES)), trace=trace
    )
    LAST_RESULT = res

    out = np.empty((N_NODES, D), np.float32)
    for c in range(N_CORES):
        oT = res.results[c]["outT"].astype(np.float32)
        out[c * C_DEG0:(c + 1) * C_DEG0] = oT[:, :C_DEG0].T
        for d in range(1, MAX_DEG + 1):
            base = P_DEG0 + (d - 1) * P_DEG
            gs = N_DEG0 + (d - 1) * N_PER_DEG + c * C_DEG
            out[gs:gs + C_DEG] = oT[:, base:base + C_DEG].T
    return out
